# revision 2
# baseline (speedup 1.0000x reference)
"""Trainium2 Bass kernel for BatteryMoEFlattenIntraCycleMoELayer.

out[b] = sum_{e in top2(b)} gate[b,e] * (x[b] @ W_e.T + bias_e),  cast to bf16

Strategy: data-parallel over B across 8 cores (16 samples/core).
Per core, on device:
  - gating: unnormalized softmax numerator (the denominator cancels in the
    top-2 renormalization; logits are O(5) so exp is safe), mask, top-2 via
    vector max/max_index, renormalize with the eps term scaled by Z.
  - gating scalars are assembled in a [32,32] tile (samples on partitions:
    col0=g0, col1/2 = expert column offsets as u32 bit patterns) plus a
    second tile with col0=g1; one DVE block-transpose each flips them so
    PE TensorLoads (exempt from partition-start rules) read the offset rows
    directly into PE registers, and partition_broadcast (a Pool-engine
    compute op, deliberately no SWDGE DMA whose transfer would queue behind
    the bulk HBM traffic) broadcasts the gate rows to 128 partitions.
  - top-2 dispatch as matmuls with the expert chosen at runtime via a
    PE-register dynamic access-pattern offset; gates folded in by
    pre-scaling x on DVE (two scaled copies per sample so PSUM evictions
    are dependency-free plain copies on the Act engine).
  - two waves of 8 samples (8 PSUM banks each); wave 0 is k-major so the
    PE burst chases the W-chunk DMA arrivals, wave 1 sample-major.
  - input DMA split across both HWDGE rings (each W chunk in two halves);
    output DMA configs kept on the SP ring and PSUM evictions on Act so
    neither queue head-of-line-blocks the other.
  - for multi-repeat (timing) NEFFs, tile pools are hoisted out of the
    repeat loop with fixed tags so consecutive bodies pipeline without
    pool-teardown write-after-read stalls; W chunks 0-3 double-buffered.
All matmul data is bf16 (fp32 matmul runs at 1/4 rate on PE); accumulation
is fp32 in PSUM; output cast to bf16 on eviction.
"""

import numpy as np
import ml_dtypes
from contextlib import ExitStack

import concourse.bass as bass
import concourse.bacc as bacc
import concourse.mybir as mybir
import concourse.tile as tile
from concourse.bass_utils import run_bass_kernel_spmd

# problem shape (hardcoded per contract)
B, L, C, CURVE = 128, 100, 3, 300
F = C * CURVE            # 900
E, D, TOPK = 8, 512, 2
EPS = 1e-9

NCORES = 8
BL = B // NCORES         # 16 samples per core
KT = 8                   # contraction tiles of 128 (900+bias row padded to 1024)
FP = KT * 128            # 1024
WAVE = 8                 # samples in flight (one PSUM bank each)
XCH = 4                  # samples per x-DMA chunk

BF16 = mybir.dt.bfloat16
F32 = mybir.dt.float32
I32 = mybir.dt.int32
U32 = mybir.dt.uint32

_BF = ml_dtypes.bfloat16

_NC_CACHE = {}


def _make_pools(tc, ctx):
    return dict(
        gp=ctx.enter_context(tc.tile_pool(name="gating", bufs=2)),
        wp=ctx.enter_context(tc.tile_pool(name="wpool", bufs=1)),
        wpa=ctx.enter_context(tc.tile_pool(name="wpoolA", bufs=2)),
        xp=ctx.enter_context(tc.tile_pool(name="xpool", bufs=XCH)),
        xsp=ctx.enter_context(tc.tile_pool(name="xspool", bufs=2 * BL)),
        pp=ctx.enter_context(tc.tile_pool(name="psum", bufs=WAVE,
                                          space="PSUM")),
        op=ctx.enter_context(tc.tile_pool(name="outp", bufs=WAVE)),
    )


def _emit_body(nc, tc, pools, xh, wh, lg, mk, out, R=""):
    PE = mybir.EngineType.PE
    H = E * D // 2
    gp, wp, wpa = pools["gp"], pools["wp"], pools["wpa"]
    xp, xsp, pp, op = pools["xp"], pools["xsp"], pools["pp"], pools["op"]

    lg_sb = gp.tile([BL, E], F32, tag="lg", name=f"{R}lg_sb")
    mk_sb = gp.tile([BL, E], I32, tag="mk", name=f"{R}mk_sb")
    w_t = [(wpa if k < 4 else wp).tile([128, E * D], BF16, tag=f"w{k}",
                                       name=f"{R}w{k}")
           for k in range(KT)]
    x_t = [xp.tile([128, XCH * KT * L], BF16, tag="xch", name=f"{R}xch{c}")
           for c in range(BL // XCH)]

    r32 = gp.tile([32, 32], F32, tag="r32", name=f"{R}r32")
    r32b = gp.tile([32, 32], F32, tag="r32b", name=f"{R}r32b")
    nc.vector.memset(r32, 0.0)
    nc.vector.memset(r32b, 0.0)

    def _load_x(c, eng):
        eng.dma_start(x_t[c], xh[:, c * XCH * KT * L:(c + 1) * XCH * KT * L])

    def _load_w(k):
        nc.sync.dma_start(w_t[k][:, 0:H], wh[k][:, 0:H])
        nc.scalar.dma_start(w_t[k][:, H:2 * H], wh[k][:, H:2 * H])

    nc.sync.dma_start(lg_sb, lg[:, :])
    nc.sync.dma_start(mk_sb, mk[:, :])
    nc.sync.dma_start(x_t[0], xh[:, 0:XCH * KT * L])
    _load_w(0)

    p_t = gp.tile([BL, E], F32, tag="p_t", name=f"{R}p_t")
    z_t = gp.tile([BL, 1], F32, tag="z_t", name=f"{R}z_t")
    nc.scalar.activation(p_t, lg_sb, mybir.ActivationFunctionType.Exp,
                         bias=0.0, scale=1.0, accum_out=z_t)

    _load_x(1, nc.scalar)
    _load_w(1)
    _load_w(2)
    _load_x(2, nc.scalar)
    for k in range(3, 5):
        _load_w(k)
    _load_x(3, nc.scalar)
    for k in range(5, KT):
        _load_w(k)

    mf = gp.tile([BL, E], F32, tag="mf", name=f"{R}mf")
    nc.vector.tensor_copy(mf, mk_sb)
    g_t = gp.tile([BL, E], F32, tag="g_t", name=f"{R}g_t")
    nc.vector.tensor_tensor(g_t, p_t, mf, mybir.AluOpType.mult)

    max8 = gp.tile([BL, 8], F32, tag="max8", name=f"{R}max8")
    idx8 = gp.tile([BL, 8], U32, tag="idx8", name=f"{R}idx8")
    nc.vector.max(max8, g_t)
    nc.vector.max_index(idx8, max8, g_t)

    s0 = gp.tile([BL, 1], F32, tag="s0", name=f"{R}s0")
    nc.vector.tensor_tensor(s0, max8[:, 0:1], max8[:, 1:2], mybir.AluOpType.add)
    s1 = gp.tile([BL, 1], F32, tag="s1", name=f"{R}s1")
    nc.vector.scalar_tensor_tensor(s1, z_t, EPS, s0,
                                   mybir.AluOpType.mult, mybir.AluOpType.add)
    r_t = gp.tile([BL, 1], F32, tag="r_t", name=f"{R}r_t")
    nc.vector.reciprocal(r_t, s1)

    # r32: col0=g0, cols1-2 = expert offsets as u32 bit patterns (the DVE
    # transpose is a byte shuffle so they survive bit-exact; TensorLoad is
    # exempt from partition-start rules and reads transposed rows 1-2
    # directly). r32b: col0=g1 so its transposed row 0 sits at partition 0
    # for the broadcast.
    nc.vector.tensor_tensor(r32[0:BL, 0:1], max8[:, 0:1], r_t,
                            mybir.AluOpType.mult)
    nc.vector.tensor_scalar(r32[0:BL, 1:3].bitcast(U32), idx8[:, 0:2], 9,
                            None, mybir.AluOpType.logical_shift_left)
    nc.vector.tensor_tensor(r32b[0:BL, 0:1], max8[:, 1:2], r_t,
                            mybir.AluOpType.mult)

    t32 = gp.tile([32, 32], F32, tag="t32", name=f"{R}t32")
    t32b = gp.tile([32, 32], F32, tag="t32b", name=f"{R}t32b")
    nc.vector.transpose(t32, r32)
    nc.vector.transpose(t32b, r32b)

    gbc0 = gp.tile([128, BL], F32, tag="gbc0", name=f"{R}gbc0")
    gbc1 = gp.tile([128, BL], F32, tag="gbc1", name=f"{R}gbc1")
    nc.gpsimd.partition_broadcast(gbc0, t32[0:1, 0:BL])
    nc.gpsimd.partition_broadcast(gbc1, t32b[0:1, 0:BL])

    xs_t = {}
    for b in range(BL):
        ch = x_t[b // XCH]
        src = ch[:, (b % XCH) * KT * L:(b % XCH + 1) * KT * L]
        for slot, gbc in ((0, gbc0), (1, gbc1)):
            xs = xsp.tile([128, KT * L], BF16, tag="xs",
                          name=f"{R}xs{b}_{slot}")
            nc.vector.tensor_scalar_mul(xs, src, gbc[:, b:b + 1])
            xs_t[(b, slot)] = xs

    _, off0 = nc.values_load_multi_w_load_instructions(
        t32[1:2, 0:BL].bitcast(I32), engines=(PE,),
        min_val=0, max_val=(E - 1) * D, skip_runtime_bounds_check=True)
    _, off1 = nc.values_load_multi_w_load_instructions(
        t32[2:3, 0:BL].bitcast(I32), engines=(PE,),
        min_val=0, max_val=(E - 1) * D, skip_runtime_bounds_check=True)
    offs = {}
    for b in range(BL):
        offs[(b, 0)] = off0[b]
        offs[(b, 1)] = off1[b]

    for wave in range(BL // WAVE):
        psums = [pp.tile([L, D], F32, tag="ps", name=f"{R}ps{wave}_{j}")
                 for j in range(WAVE)]
        if wave == 0:
            order = [(k, j) for k in range(KT) for j in range(WAVE)]
        else:
            order = [(k, j) for j in range(WAVE) for k in range(KT)]
        for k, j in order:
            b = wave * WAVE + j
            for slot in range(TOPK):
                nc.tensor.matmul(
                    psums[j],
                    xs_t[(b, slot)][:, k * L:(k + 1) * L],
                    w_t[k][:, bass.ds(offs[(b, slot)], D)],
                    start=(k == 0 and slot == 0),
                    stop=(k == KT - 1 and slot == TOPK - 1),
                )
        for j in range(WAVE):
            b = wave * WAVE + j
            ot = op.tile([L, D], BF16, tag="ot", name=f"{R}ot{b}")
            nc.scalar.activation(ot, psums[j],
                                 mybir.ActivationFunctionType.Copy)
            nc.sync.dma_start(out[b], ot)


def _build_nc(repeats=1):
    nc = bacc.Bacc("TRN2", target_bir_lowering=False)

    xh = nc.declare_dram_parameter("xh", [128, BL * KT * L], BF16, isOutput=False)
    wh = nc.declare_dram_parameter("wh", [KT, 128, E * D], BF16, isOutput=False)
    lg = nc.declare_dram_parameter("lg", [BL, E], F32, isOutput=False)
    mk = nc.declare_dram_parameter("mk", [BL, E], I32, isOutput=False)
    out = nc.declare_dram_parameter("out", [BL, L, D], BF16, isOutput=True)

    with tile.TileContext(nc) as tc, ExitStack() as ctx:
        pools = _make_pools(tc, ctx)
        for rep in range(repeats):
            R = f"r{rep}_" if repeats > 1 else ""
            _emit_body(nc, tc, pools, xh, wh, lg, mk, out, R=R)

    nc.compile()
    return nc


def get_nc(repeats=1):
    key = ("nc", repeats)
    if key not in _NC_CACHE:
        _NC_CACHE[key] = _build_nc(repeats)
    return _NC_CACHE[key]


def _prep_w(W, b):
    """-> [KT, 128, E*D] bf16: wh[k, p, e, d] = Wt_pad[e, 128k+p, d] where
    Wt_pad = [W_e^T (900 rows); bias_e (row 900); zeros (rows 901..1023)]."""
    wt = np.zeros((E, FP, D), np.float32)
    wt[:, :F, :] = np.asarray(W, np.float32).transpose(0, 2, 1)
    wt[:, F, :] = np.asarray(b, np.float32)
    wh = wt.reshape(E, KT, 128, D).transpose(1, 2, 0, 3).reshape(KT, 128, E * D)
    return np.ascontiguousarray(wh).astype(_BF)


def _prep_x(x):
    """-> [128, B, KT*L] bf16: xh[p, b, k*L+l] = xt_pad[b, 128k+p, l] where
    xt_pad = [x_b^T (900 rows); ones (row 900); zeros]."""
    x = np.asarray(x, np.float32).reshape(B, L, F)
    xt = np.zeros((B, FP, L), np.float32)
    xt[:, :F, :] = x.transpose(0, 2, 1)
    xt[:, F, :] = 1.0
    xh = xt.reshape(B, KT, 128, L).transpose(2, 0, 1, 3).reshape(128, B, KT * L)
    return np.ascontiguousarray(xh).astype(_BF)


LAST_RESULT = None


def kernel(cycle_curve_data, logits, moe_masks, W, b):
    global LAST_RESULT
    nc = get_nc()

    wh = _prep_w(W, b)
    xh = _prep_x(cycle_curve_data)
    lg = np.ascontiguousarray(np.asarray(logits, np.float32))
    mk = np.ascontiguousarray(np.asarray(moe_masks, np.int32))

    in_maps = []
    for c in range(NCORES):
        s = slice(c * BL, (c + 1) * BL)
        in_maps.append({
            "xh": np.ascontiguousarray(xh[:, s].reshape(128, BL * KT * L)),
            "wh": wh,
            "lg": np.ascontiguousarray(lg[s]),
            "mk": np.ascontiguousarray(mk[s]),
        })

    res = run_bass_kernel_spmd(nc, in_maps, core_ids=list(range(NCORES)))
    LAST_RESULT = res
    outs = [np.asarray(r["out"]) for r in res.results]
    return np.concatenate(outs, axis=0)
